# revision 22
# baseline (speedup 1.0000x reference)
"""DeepSeek-mini MoE block on 8 Trainium2 NeuronCores.

Strategy (expert-parallel with token dispatch, per the sharding hint):
  - Host computes rmsnorm + router (softmax/top-2/dispatch weights) and the
    balance scalar -- O(T*H*E) work, negligible next to the FFN GEMMs.
  - Core c receives expert c's FFN weights (pre-transposed into matmul lhsT
    tile layout, cast bf16), the tokens routed to expert c (gathered,
    transposed, padded to a common capacity C), the full shared-expert
    weights, and its own T/8 = 512-token slice for the shared expert
    (data-parallel shared expert: every token's shared FFN is computed by
    exactly one core, so no cross-core partial sums are needed).
  - Device runs one uniform SwiGLU pipeline over two weight/token streams
    (shared stream first -- its input DMA is small, which hides startup; the
    expert stream's bigger gather prefetches during shared compute).
    Everything stays in "transposed activation" space (activations
    [H|F, tokens]) so no on-chip transposes are needed.
  - Device pre-scales the routed output columns by the dispatch weights
    (shared columns by 1.0); host scatter-adds into residual + done.
"""

import os
import sys

for _p in ("/opt/trn_rl_repo", "/opt/pypackages"):
    if os.path.isdir(_p) and _p not in sys.path:
        sys.path.append(_p)

import numpy as np
import ml_dtypes

import concourse.bass as bass  # noqa: F401  (engine registration)
import concourse.mybir as mybir
import concourse.tile as tile
from concourse import bacc
from concourse.bass_utils import run_bass_kernel_spmd

BF16 = ml_dtypes.bfloat16
AF = mybir.ActivationFunctionType
ALU = mybir.AluOpType

B, S, H, F, E, TOPK = 2, 2048, 2048, 4096, 8, 2
T = B * S
EPS = 1e-6
BALANCE_FACTOR = 1e-4
NCORES = 8
P = 128
NCHUNK = 512   # token columns per matmul (one PSUM bank of fp32)
SCMAX = 1280   # routed-token super-chunk cap (SBUF bound)
TSH = T // NCORES  # shared-expert tokens per core (512)

KH = H // P   # 16 K-tiles over H
KF = F // P   # 32 K-tiles over F
MF = F // P   # 32 M-tiles over F
MH = H // P   # 16 M-tiles over H

LAST_RESULTS = {}


def _chunks(total, size):
    """Split `total` columns into near-equal chunks of at most `size`.

    Near-equal (rather than size,size,...,remainder) keeps every chunk well
    above 128 columns: a matmul with N < 128 is LDWEIGHTS-bound (~128 cycles
    regardless of N), so a skinny remainder chunk wastes PE time.
    """
    n = -(-total // size)
    base, extra = divmod(total, n)
    out, off = [], 0
    for i in range(n):
        ln = base + (1 if i < extra else 0)
        out.append((off, ln))
        off += ln
    return out


def _build_program(C):
    dt = mybir.dt
    CT = C + TSH
    nc = bacc.Bacc("TRN2", target_bir_lowering=False, debug=False)

    xs = nc.dram_tensor("xs", [P, KH, TSH], dt.bfloat16, kind="ExternalInput")
    xgt = nc.dram_tensor("xgt", [P, KH, C], dt.bfloat16, kind="ExternalInput")
    w1 = nc.dram_tensor("w1", [MF, P, KH, P], dt.bfloat16, kind="ExternalInput")
    w3 = nc.dram_tensor("w3", [MF, P, KH, P], dt.bfloat16, kind="ExternalInput")
    w2 = nc.dram_tensor("w2", [MH, P, KF, P], dt.bfloat16, kind="ExternalInput")
    v1 = nc.dram_tensor("v1", [MF, P, KH, P], dt.bfloat16, kind="ExternalInput")
    v3 = nc.dram_tensor("v3", [MF, P, KH, P], dt.bfloat16, kind="ExternalInput")
    v2 = nc.dram_tensor("v2", [MH, P, KF, P], dt.bfloat16, kind="ExternalInput")
    wb = nc.dram_tensor("wb", [P, CT], dt.float32, kind="ExternalInput")
    yout = nc.dram_tensor("yout", [MH, P, CT], dt.float32, kind="ExternalOutput")

    with tile.TileContext(nc) as tc:
        with tc.tile_pool(name="const", bufs=1) as pconst, \
             tc.tile_pool(name="xg", bufs=1) as pxg, \
             tc.tile_pool(name="gt", bufs=1) as pgt, \
             tc.tile_pool(name="wstream", bufs=3) as pw, \
             tc.tile_pool(name="evict", bufs=4) as pev, \
             tc.tile_pool(name="psA", bufs=4, space="PSUM") as pps, \
             tc.tile_pool(name="psB", bufs=4, space="PSUM") as pps2:
            # PE warmup: ~10us of dependency-free matmuls on const tiles keeps
            # the HAM clock gate warm (2.4 GHz) through the DMA-latency window
            # at kernel start, so the first real matmuls run at full rate.
            wz = pconst.tile([P, 2 * P], dt.bfloat16, tag="warm")
            # only a 1-column token write: the warmup matmuls may read garbage
            # (their PSUM is never consumed); a full memset would chain the
            # first PE instruction behind another engine's startup
            nc.gpsimd.memset(wz[:, :1], 0.0)
            pswarm = pps.tile([P, P], dt.float32, tag="ps")
            NWARM = 200
            for i in range(NWARM):
                nc.tensor.matmul(pswarm[:], wz[:, :P], wz[:, P:],
                                 start=(i == 0), stop=(i == NWARM - 1))

            wbt = pconst.tile([P, CT], dt.float32, tag="wb")
            wb_loaded = False

            # (input handle, weight handles, group token count, column offset,
            #  xg tag) -- shared stream first: small input DMA hides startup.
            groups = [
                (xs, v1, v3, v2, TSH, C, "xgs"),
                (xgt, w1, w3, w2, C, 0, "xge"),
            ]
            for xh, g1, g3, g2, CG, coff, xtag in groups:
                for sc0, scn in _chunks(CG, SCMAX):
                    xg_t = pxg.tile([P, KH, scn], dt.bfloat16, tag=xtag)
                    # split along K so the first matmuls start before the
                    # whole token tile has landed
                    for kq in range(0, KH, 4):
                        nc.sync.dma_start(xg_t[:, kq:kq + 4],
                                          xh[:, kq:kq + 4, sc0:sc0 + scn])
                    gt = pgt.tile([P, KF, scn], dt.bfloat16, tag="gt")
                    jchunks = _chunks(scn, NCHUNK)
                    for m in range(MF):
                        w1m = pw.tile([P, KH, P], dt.bfloat16, tag="w1")
                        nc.sync.dma_start(w1m[:], g1[m])
                        w3m = pw.tile([P, KH, P], dt.bfloat16, tag="w3")
                        nc.sync.dma_start(w3m[:], g3[m])
                        for j0, jn in jchunks:
                            ps1 = pps.tile([P, jn], dt.float32, tag="ps")
                            for k in range(KH):
                                nc.tensor.matmul(ps1[:], w1m[:, k, :],
                                                 xg_t[:, k, j0:j0 + jn],
                                                 start=(k == 0), stop=(k == KH - 1))
                            ps3 = pps.tile([P, jn], dt.float32, tag="ps")
                            for k in range(KH):
                                nc.tensor.matmul(ps3[:], w3m[:, k, :],
                                                 xg_t[:, k, j0:j0 + jn],
                                                 start=(k == 0), stop=(k == KH - 1))
                            sil = pev.tile([P, jn], dt.float32, tag="sil")
                            nc.scalar.activation(sil[:], ps1[:], AF.Silu)
                            nc.vector.tensor_tensor(out=gt[:, m, j0:j0 + jn],
                                                    in0=sil[:], in1=ps3[:],
                                                    op=ALU.mult)
                    if not wb_loaded:
                        # deferred off the startup critical path; first use is
                        # the first A2 eviction, hundreds of us in
                        nc.sync.dma_start(wbt[:], wb[:])
                        wb_loaded = True
                    for h in range(MH):
                        w2h = pw.tile([P, KF, P], dt.bfloat16, tag="w2")
                        nc.sync.dma_start(w2h[:], g2[h])
                        for j0, jn in jchunks:
                            ps = pps2.tile([P, jn], dt.float32, tag="ps2")
                            for k2 in range(KF):
                                nc.tensor.matmul(ps[:], w2h[:, k2, :],
                                                 gt[:, k2, j0:j0 + jn],
                                                 start=(k2 == 0), stop=(k2 == KF - 1))
                            eo = pev.tile([P, jn], dt.float32, tag="eo")
                            c0 = coff + sc0 + j0
                            nc.vector.tensor_tensor(out=eo[:], in0=ps[:],
                                                    in1=wbt[:, c0:c0 + jn],
                                                    op=ALU.mult)
                            nc.sync.dma_start(yout[h, :, c0:c0 + jn], eo[:])

    nc.finalize()
    return nc


def _fmt_lhsT(w, mtiles, ktiles):
    """[M, K] row-major -> [mtiles, P(kpart), ktiles, P(mcol)] bf16 lhsT blocks."""
    return np.ascontiguousarray(
        w.reshape(mtiles, P, ktiles, P).transpose(0, 3, 2, 1).astype(BF16))


def kernel(**inputs):
    hs = np.asarray(inputs["hidden_states"], dtype=np.float32)
    norm_w = np.asarray(inputs["norm_w"], dtype=np.float32)
    router_w = np.asarray(inputs["router_w"], dtype=np.float32)
    expert_bias = np.asarray(inputs["expert_bias"], dtype=np.float32)
    sw1 = np.asarray(inputs["sw1"], dtype=np.float32)
    sw2 = np.asarray(inputs["sw2"], dtype=np.float32)
    sw3 = np.asarray(inputs["sw3"], dtype=np.float32)
    ew1 = np.asarray(inputs["ew1"], dtype=np.float32)
    ew2 = np.asarray(inputs["ew2"], dtype=np.float32)
    ew3 = np.asarray(inputs["ew3"], dtype=np.float32)

    # ---- host: rmsnorm + router + dispatch (the "all-to-all" sharding layer)
    hflat = hs.reshape(T, H)
    ms = np.mean(hflat * hflat, axis=1, keepdims=True, dtype=np.float32)
    x = (hflat / np.sqrt(ms + EPS)) * norm_w

    logits = x @ router_w.T + expert_bias
    lmax = logits.max(axis=1, keepdims=True)
    ex = np.exp(logits - lmax, dtype=np.float32)
    probs = ex / ex.sum(axis=1, keepdims=True)
    order = np.argsort(-probs, axis=1, kind="stable")
    topi = order[:, :TOPK]
    topv = np.take_along_axis(probs, topi, axis=1)
    topv = topv / topv.sum(axis=1, keepdims=True)
    disp = np.zeros((T, E), np.float32)
    np.put_along_axis(disp, topi, topv.astype(np.float32), axis=1)

    load = probs.mean(axis=0, dtype=np.float32)
    target = np.float32(1.0 / E)
    balance = np.float32(
        np.sum(target * (np.log(target) - np.log(load))) / E * BALANCE_FACTOR)

    idxs, wvals = [], []
    for c in range(E):
        mask = (topi == c).any(axis=1)
        idx = np.nonzero(mask)[0]
        idxs.append(idx)
        wvals.append(disp[idx, c])
    maxn = max(len(i) for i in idxs)
    C = max(((maxn + 1) // 2) * 2, 32)
    CT = C + TSH

    xb = x.astype(BF16)
    # xt_host[j, p, k, t] = x[j*TSH + t, k*P + p] -- core c's shared slice is [c]
    xt_host = np.ascontiguousarray(
        xb.reshape(NCORES, TSH, KH, P).transpose(0, 3, 2, 1))

    # shared-expert weights: identical for every core, reformat once
    v1_host = _fmt_lhsT(sw1, MF, KH)
    v3_host = _fmt_lhsT(sw3, MF, KH)
    v2_host = _fmt_lhsT(sw2, MH, KF)

    in_maps = []
    for c in range(E):
        idx = idxs[c]
        pad = np.zeros(C, np.int64)
        pad[:len(idx)] = idx
        xg = xb[pad]  # [C, H] bf16 gather
        xgt_host = np.ascontiguousarray(xg.reshape(C, KH, P).transpose(2, 1, 0))
        wcol = np.ones(CT, np.float32)
        wcol[:C] = 0.0
        wcol[:len(idx)] = wvals[c]
        wb_host = np.ascontiguousarray(np.broadcast_to(wcol, (P, CT)))
        in_maps.append({
            "xs": xt_host[c],
            "xgt": xgt_host,
            "w1": _fmt_lhsT(ew1[c], MF, KH),
            "w3": _fmt_lhsT(ew3[c], MF, KH),
            "w2": _fmt_lhsT(ew2[c], MH, KF),
            "v1": v1_host,
            "v3": v3_host,
            "v2": v2_host,
            "wb": wb_host,
        })

    nc = _build_program(C)
    trace = os.environ.get("KERNEL_TRACE", "0") == "1"
    try:
        res = run_bass_kernel_spmd(nc, in_maps, core_ids=list(range(NCORES)),
                                   trace=trace)
    except Exception:
        # transient NRT_EXEC_UNIT_UNRECOVERABLE wedges resolve on retry
        res = run_bass_kernel_spmd(nc, in_maps, core_ids=list(range(NCORES)),
                                   trace=False)
    LAST_RESULTS["exec_time_ns"] = res.exec_time_ns
    LAST_RESULTS["profile_json"] = res.profile_json
    LAST_RESULTS["trace"] = res.instructions_and_trace
    LAST_RESULTS["C"] = C

    # ---- host: unshard. yout columns [0:C] = routed (pre-scaled), [C:] = shared
    out = hflat.copy()
    for c in range(E):
        y = res.results[c]["yout"].transpose(2, 0, 1).reshape(CT, H)
        out[c * TSH:(c + 1) * TSH] += y[C:]
        n = len(idxs[c])
        if n:
            out[idxs[c]] += y[:n]
    return out.reshape(B, S, H), balance


# revision 23
# speedup vs baseline: 1.2015x; 1.2015x over previous
"""DeepSeek-mini MoE block on 8 Trainium2 NeuronCores.

Strategy (expert-parallel with token dispatch, per the sharding hint):
  - Host computes rmsnorm + router (softmax/top-2/dispatch weights) and the
    balance scalar -- O(T*H*E) work, negligible next to the FFN GEMMs.
  - Core c receives expert c's FFN weights (pre-transposed into matmul lhsT
    tile layout, cast bf16), the tokens routed to expert c (gathered,
    transposed, padded to a common capacity C), the full shared-expert
    weights, and its own T/8 = 512-token slice for the shared expert
    (data-parallel shared expert: every token's shared FFN is computed by
    exactly one core, so no cross-core partial sums are needed).
  - Device runs one uniform SwiGLU pipeline over two weight/token streams
    (shared stream first -- its input DMA is small, which hides startup; the
    expert stream's bigger gather prefetches during shared compute).
    Everything stays in "transposed activation" space (activations
    [H|F, tokens]) so no on-chip transposes are needed.
  - Device pre-scales the routed output columns by the dispatch weights
    (shared columns by 1.0); host scatter-adds into residual + done.
"""

import os
import sys

for _p in ("/opt/trn_rl_repo", "/opt/pypackages"):
    if os.path.isdir(_p) and _p not in sys.path:
        sys.path.append(_p)

import numpy as np
import ml_dtypes

import concourse.bass as bass  # noqa: F401  (engine registration)
import concourse.mybir as mybir
import concourse.tile as tile
from concourse import bacc
from concourse.bass_utils import run_bass_kernel_spmd

BF16 = ml_dtypes.bfloat16
AF = mybir.ActivationFunctionType
ALU = mybir.AluOpType

B, S, H, F, E, TOPK = 2, 2048, 2048, 4096, 8, 2
T = B * S
EPS = 1e-6
BALANCE_FACTOR = 1e-4
NCORES = 8
P = 128
NCHUNK = 512   # token columns per matmul (one PSUM bank of fp32)
SCMAX = 1280   # routed-token super-chunk cap (SBUF bound)
TSH = T // NCORES  # shared-expert tokens per core (512)

KH = H // P   # 16 K-tiles over H
KF = F // P   # 32 K-tiles over F
MF = F // P   # 32 M-tiles over F
MH = H // P   # 16 M-tiles over H

LAST_RESULTS = {}


def _chunks(total, size):
    """Split `total` columns into near-equal chunks of at most `size`.

    Near-equal (rather than size,size,...,remainder) keeps every chunk well
    above 128 columns: a matmul with N < 128 is LDWEIGHTS-bound (~128 cycles
    regardless of N), so a skinny remainder chunk wastes PE time.
    """
    n = -(-total // size)
    base, extra = divmod(total, n)
    out, off = [], 0
    for i in range(n):
        ln = base + (1 if i < extra else 0)
        out.append((off, ln))
        off += ln
    return out


def _build_program(C):
    dt = mybir.dt
    CT = C + TSH
    nc = bacc.Bacc("TRN2", target_bir_lowering=False, debug=False)

    xs = nc.dram_tensor("xs", [P, KH, TSH], dt.bfloat16, kind="ExternalInput")
    xgt = nc.dram_tensor("xgt", [P, KH, C], dt.bfloat16, kind="ExternalInput")
    w1 = nc.dram_tensor("w1", [MF, P, KH, P], dt.bfloat16, kind="ExternalInput")
    w3 = nc.dram_tensor("w3", [MF, P, KH, P], dt.bfloat16, kind="ExternalInput")
    w2 = nc.dram_tensor("w2", [MH, P, KF, P], dt.bfloat16, kind="ExternalInput")
    v1 = nc.dram_tensor("v1", [MF, P, KH, P], dt.bfloat16, kind="ExternalInput")
    v3 = nc.dram_tensor("v3", [MF, P, KH, P], dt.bfloat16, kind="ExternalInput")
    v2 = nc.dram_tensor("v2", [MH, P, KF, P], dt.bfloat16, kind="ExternalInput")
    wb = nc.dram_tensor("wb", [P, CT], dt.float32, kind="ExternalInput")
    yout = nc.dram_tensor("yout", [MH, P, CT], dt.float32, kind="ExternalOutput")

    with tile.TileContext(nc) as tc:
        with tc.tile_pool(name="const", bufs=1) as pconst, \
             tc.tile_pool(name="xg", bufs=1) as pxg, \
             tc.tile_pool(name="gt", bufs=1) as pgt, \
             tc.tile_pool(name="wstream", bufs=3) as pw, \
             tc.tile_pool(name="evict", bufs=4) as pev, \
             tc.tile_pool(name="psA", bufs=4, space="PSUM") as pps, \
             tc.tile_pool(name="psB", bufs=4, space="PSUM") as pps2:
            # PE warmup: ~10us of dependency-free matmuls on const tiles keeps
            # the HAM clock gate warm (2.4 GHz) through the DMA-latency window
            # at kernel start, so the first real matmuls run at full rate.
            wz = pconst.tile([P, 2 * P], dt.bfloat16, tag="warm")
            nc.gpsimd.memset(wz[:], 0.0)
            pswarm = pps.tile([P, P], dt.float32, tag="ps")
            NWARM = 176
            for i in range(NWARM):
                nc.tensor.matmul(pswarm[:], wz[:, :P], wz[:, P:],
                                 start=(i == 0), stop=(i == NWARM - 1))

            wbt = pconst.tile([P, CT], dt.float32, tag="wb")
            wb_loaded = False

            # (input handle, weight handles, group token count, column offset,
            #  xg tag) -- shared stream first: small input DMA hides startup.
            groups = [
                (xs, v1, v3, v2, TSH, C, "xgs"),
                (xgt, w1, w3, w2, C, 0, "xge"),
            ]
            for xh, g1, g3, g2, CG, coff, xtag in groups:
                for sc0, scn in _chunks(CG, SCMAX):
                    xg_t = pxg.tile([P, KH, scn], dt.bfloat16, tag=xtag)
                    # split along K so the first matmuls start before the
                    # whole token tile has landed
                    for kq in range(0, KH, 4):
                        nc.sync.dma_start(xg_t[:, kq:kq + 4],
                                          xh[:, kq:kq + 4, sc0:sc0 + scn])
                    gt = pgt.tile([P, KF, scn], dt.bfloat16, tag="gt")
                    jchunks = _chunks(scn, NCHUNK)
                    for m in range(MF):
                        w1m = pw.tile([P, KH, P], dt.bfloat16, tag="w1")
                        nc.sync.dma_start(w1m[:], g1[m])
                        w3m = pw.tile([P, KH, P], dt.bfloat16, tag="w3")
                        nc.sync.dma_start(w3m[:], g3[m])
                        for j0, jn in jchunks:
                            ps1 = pps.tile([P, jn], dt.float32, tag="ps")
                            for k in range(KH):
                                nc.tensor.matmul(ps1[:], w1m[:, k, :],
                                                 xg_t[:, k, j0:j0 + jn],
                                                 start=(k == 0), stop=(k == KH - 1))
                            ps3 = pps.tile([P, jn], dt.float32, tag="ps")
                            for k in range(KH):
                                nc.tensor.matmul(ps3[:], w3m[:, k, :],
                                                 xg_t[:, k, j0:j0 + jn],
                                                 start=(k == 0), stop=(k == KH - 1))
                            sil = pev.tile([P, jn], dt.float32, tag="sil")
                            nc.scalar.activation(sil[:], ps1[:], AF.Silu)
                            nc.vector.tensor_tensor(out=gt[:, m, j0:j0 + jn],
                                                    in0=sil[:], in1=ps3[:],
                                                    op=ALU.mult)
                    if not wb_loaded:
                        # deferred off the startup critical path; first use is
                        # the first A2 eviction, hundreds of us in
                        nc.sync.dma_start(wbt[:], wb[:])
                        wb_loaded = True
                    for h in range(MH):
                        w2h = pw.tile([P, KF, P], dt.bfloat16, tag="w2")
                        nc.sync.dma_start(w2h[:], g2[h])
                        for j0, jn in jchunks:
                            ps = pps2.tile([P, jn], dt.float32, tag="ps2")
                            for k2 in range(KF):
                                nc.tensor.matmul(ps[:], w2h[:, k2, :],
                                                 gt[:, k2, j0:j0 + jn],
                                                 start=(k2 == 0), stop=(k2 == KF - 1))
                            eo = pev.tile([P, jn], dt.float32, tag="eo")
                            c0 = coff + sc0 + j0
                            nc.vector.tensor_tensor(out=eo[:], in0=ps[:],
                                                    in1=wbt[:, c0:c0 + jn],
                                                    op=ALU.mult)
                            nc.sync.dma_start(yout[h, :, c0:c0 + jn], eo[:])

    nc.finalize()
    return nc


def _fmt_lhsT(w, mtiles, ktiles):
    """[M, K] row-major -> [mtiles, P(kpart), ktiles, P(mcol)] bf16 lhsT blocks."""
    return np.ascontiguousarray(
        w.reshape(mtiles, P, ktiles, P).transpose(0, 3, 2, 1).astype(BF16))


def kernel(**inputs):
    hs = np.asarray(inputs["hidden_states"], dtype=np.float32)
    norm_w = np.asarray(inputs["norm_w"], dtype=np.float32)
    router_w = np.asarray(inputs["router_w"], dtype=np.float32)
    expert_bias = np.asarray(inputs["expert_bias"], dtype=np.float32)
    sw1 = np.asarray(inputs["sw1"], dtype=np.float32)
    sw2 = np.asarray(inputs["sw2"], dtype=np.float32)
    sw3 = np.asarray(inputs["sw3"], dtype=np.float32)
    ew1 = np.asarray(inputs["ew1"], dtype=np.float32)
    ew2 = np.asarray(inputs["ew2"], dtype=np.float32)
    ew3 = np.asarray(inputs["ew3"], dtype=np.float32)

    # ---- host: rmsnorm + router + dispatch (the "all-to-all" sharding layer)
    hflat = hs.reshape(T, H)
    ms = np.mean(hflat * hflat, axis=1, keepdims=True, dtype=np.float32)
    x = (hflat / np.sqrt(ms + EPS)) * norm_w

    logits = x @ router_w.T + expert_bias
    lmax = logits.max(axis=1, keepdims=True)
    ex = np.exp(logits - lmax, dtype=np.float32)
    probs = ex / ex.sum(axis=1, keepdims=True)
    order = np.argsort(-probs, axis=1, kind="stable")
    topi = order[:, :TOPK]
    topv = np.take_along_axis(probs, topi, axis=1)
    topv = topv / topv.sum(axis=1, keepdims=True)
    disp = np.zeros((T, E), np.float32)
    np.put_along_axis(disp, topi, topv.astype(np.float32), axis=1)

    load = probs.mean(axis=0, dtype=np.float32)
    target = np.float32(1.0 / E)
    balance = np.float32(
        np.sum(target * (np.log(target) - np.log(load))) / E * BALANCE_FACTOR)

    idxs, wvals = [], []
    for c in range(E):
        mask = (topi == c).any(axis=1)
        idx = np.nonzero(mask)[0]
        idxs.append(idx)
        wvals.append(disp[idx, c])
    maxn = max(len(i) for i in idxs)
    C = max(((maxn + 1) // 2) * 2, 32)
    CT = C + TSH

    xb = x.astype(BF16)
    # xt_host[j, p, k, t] = x[j*TSH + t, k*P + p] -- core c's shared slice is [c]
    xt_host = np.ascontiguousarray(
        xb.reshape(NCORES, TSH, KH, P).transpose(0, 3, 2, 1))

    # shared-expert weights: identical for every core, reformat once
    v1_host = _fmt_lhsT(sw1, MF, KH)
    v3_host = _fmt_lhsT(sw3, MF, KH)
    v2_host = _fmt_lhsT(sw2, MH, KF)

    in_maps = []
    for c in range(E):
        idx = idxs[c]
        pad = np.zeros(C, np.int64)
        pad[:len(idx)] = idx
        xg = xb[pad]  # [C, H] bf16 gather
        xgt_host = np.ascontiguousarray(xg.reshape(C, KH, P).transpose(2, 1, 0))
        wcol = np.ones(CT, np.float32)
        wcol[:C] = 0.0
        wcol[:len(idx)] = wvals[c]
        wb_host = np.ascontiguousarray(np.broadcast_to(wcol, (P, CT)))
        in_maps.append({
            "xs": xt_host[c],
            "xgt": xgt_host,
            "w1": _fmt_lhsT(ew1[c], MF, KH),
            "w3": _fmt_lhsT(ew3[c], MF, KH),
            "w2": _fmt_lhsT(ew2[c], MH, KF),
            "v1": v1_host,
            "v3": v3_host,
            "v2": v2_host,
            "wb": wb_host,
        })

    nc = _build_program(C)
    trace = os.environ.get("KERNEL_TRACE", "0") == "1"
    try:
        res = run_bass_kernel_spmd(nc, in_maps, core_ids=list(range(NCORES)),
                                   trace=trace)
    except Exception:
        # transient NRT_EXEC_UNIT_UNRECOVERABLE wedges resolve on retry
        res = run_bass_kernel_spmd(nc, in_maps, core_ids=list(range(NCORES)),
                                   trace=False)
    LAST_RESULTS["exec_time_ns"] = res.exec_time_ns
    LAST_RESULTS["profile_json"] = res.profile_json
    LAST_RESULTS["trace"] = res.instructions_and_trace
    LAST_RESULTS["C"] = C

    # ---- host: unshard. yout columns [0:C] = routed (pre-scaled), [C:] = shared
    out = hflat.copy()
    for c in range(E):
        y = res.results[c]["yout"].transpose(2, 0, 1).reshape(CT, H)
        out[c * TSH:(c + 1) * TSH] += y[C:]
        n = len(idxs[c])
        if n:
            out[idxs[c]] += y[:n]
    return out.reshape(B, S, H), balance


# revision 25
# speedup vs baseline: 1.2019x; 1.0004x over previous
"""DeepSeek-mini MoE block on 8 Trainium2 NeuronCores.

Strategy (expert-parallel with token dispatch, per the sharding hint):
  - Host computes rmsnorm + router (softmax/top-2/dispatch weights) and the
    balance scalar -- O(T*H*E) work, negligible next to the FFN GEMMs.
  - Core c receives expert c's FFN weights (pre-transposed into matmul lhsT
    tile layout, cast bf16), the tokens routed to expert c (gathered,
    transposed, padded to a common capacity C), the full shared-expert
    weights, and its own T/8 = 512-token slice for the shared expert
    (data-parallel shared expert: every token's shared FFN is computed by
    exactly one core, so no cross-core partial sums are needed).
  - Device runs one uniform SwiGLU pipeline over two weight/token streams
    (shared stream first -- its input DMA is small, which hides startup; the
    expert stream's bigger gather prefetches during shared compute).
    Everything stays in "transposed activation" space (activations
    [H|F, tokens]) so no on-chip transposes are needed.
  - Device pre-scales the routed output columns by the dispatch weights
    (shared columns by 1.0); host scatter-adds into residual + done.
"""

import os
import sys

for _p in ("/opt/trn_rl_repo", "/opt/pypackages"):
    if os.path.isdir(_p) and _p not in sys.path:
        sys.path.append(_p)

import numpy as np
import ml_dtypes

import concourse.bass as bass  # noqa: F401  (engine registration)
import concourse.mybir as mybir
import concourse.tile as tile
from concourse import bacc
from concourse.bass_utils import run_bass_kernel_spmd

BF16 = ml_dtypes.bfloat16
AF = mybir.ActivationFunctionType
ALU = mybir.AluOpType

B, S, H, F, E, TOPK = 2, 2048, 2048, 4096, 8, 2
T = B * S
EPS = 1e-6
BALANCE_FACTOR = 1e-4
NCORES = 8
P = 128
NCHUNK = 512   # token columns per matmul (one PSUM bank of fp32)
SCMAX = 1280   # routed-token super-chunk cap (SBUF bound)
TSH = T // NCORES  # shared-expert tokens per core (512)

KH = H // P   # 16 K-tiles over H
KF = F // P   # 32 K-tiles over F
MF = F // P   # 32 M-tiles over F
MH = H // P   # 16 M-tiles over H

LAST_RESULTS = {}


def _chunks(total, size):
    """Split `total` columns into near-equal chunks of at most `size`.

    Near-equal (rather than size,size,...,remainder) keeps every chunk well
    above 128 columns: a matmul with N < 128 is LDWEIGHTS-bound (~128 cycles
    regardless of N), so a skinny remainder chunk wastes PE time.
    """
    n = -(-total // size)
    base, extra = divmod(total, n)
    out, off = [], 0
    for i in range(n):
        ln = base + (1 if i < extra else 0)
        out.append((off, ln))
        off += ln
    return out


def _build_program(C):
    dt = mybir.dt
    CT = C + TSH
    nc = bacc.Bacc("TRN2", target_bir_lowering=False, debug=False)

    xs = nc.dram_tensor("xs", [P, KH, TSH], dt.bfloat16, kind="ExternalInput")
    xgt = nc.dram_tensor("xgt", [P, KH, C], dt.bfloat16, kind="ExternalInput")
    w1 = nc.dram_tensor("w1", [MF, P, KH, P], dt.bfloat16, kind="ExternalInput")
    w3 = nc.dram_tensor("w3", [MF, P, KH, P], dt.bfloat16, kind="ExternalInput")
    w2 = nc.dram_tensor("w2", [MH, P, KF, P], dt.bfloat16, kind="ExternalInput")
    v1 = nc.dram_tensor("v1", [MF, P, KH, P], dt.bfloat16, kind="ExternalInput")
    v3 = nc.dram_tensor("v3", [MF, P, KH, P], dt.bfloat16, kind="ExternalInput")
    v2 = nc.dram_tensor("v2", [MH, P, KF, P], dt.bfloat16, kind="ExternalInput")
    wb = nc.dram_tensor("wb", [P, CT], dt.float32, kind="ExternalInput")
    yout = nc.dram_tensor("yout", [MH, P, CT], dt.float32, kind="ExternalOutput")

    with tile.TileContext(nc) as tc:
        with tc.tile_pool(name="const", bufs=1) as pconst, \
             tc.tile_pool(name="xg", bufs=1) as pxg, \
             tc.tile_pool(name="gt", bufs=1) as pgt, \
             tc.tile_pool(name="wstream", bufs=3) as pw, \
             tc.tile_pool(name="evict", bufs=4) as pev, \
             tc.tile_pool(name="psA", bufs=4, space="PSUM") as pps, \
             tc.tile_pool(name="psB", bufs=4, space="PSUM") as pps2:
            # PE warmup: ~10us of dependency-free matmuls on const tiles keeps
            # the HAM clock gate warm (2.4 GHz) through the DMA-latency window
            # at kernel start, so the first real matmuls run at full rate.
            wz = pconst.tile([P, 2 * P], dt.bfloat16, tag="warm")
            nc.gpsimd.memset(wz[:], 0.0)
            pswarm = pps.tile([P, P], dt.float32, tag="ps")
            NWARM = 176
            for i in range(NWARM):
                nc.tensor.matmul(pswarm[:], wz[:, :P], wz[:, P:],
                                 start=(i == 0), stop=(i == NWARM - 1))

            wbt = pconst.tile([P, CT], dt.float32, tag="wb")
            wb_loaded = False

            # (input handle, weight handles, group token count, column offset,
            #  xg tag) -- shared stream first: small input DMA hides startup.
            groups = [
                (xs, v1, v3, v2, TSH, C, "xgs"),
                (xgt, w1, w3, w2, C, 0, "xge"),
            ]
            for xh, g1, g3, g2, CG, coff, xtag in groups:
                for sc0, scn in _chunks(CG, SCMAX):
                    xg_t = pxg.tile([P, KH, scn], dt.bfloat16, tag=xtag)
                    # split along K so the first matmuls start before the
                    # whole token tile has landed
                    for kq in range(0, KH, 4):
                        nc.sync.dma_start(xg_t[:, kq:kq + 4],
                                          xh[:, kq:kq + 4, sc0:sc0 + scn])
                    gt = pgt.tile([P, KF, scn], dt.bfloat16, tag="gt")
                    jchunks = _chunks(scn, NCHUNK)
                    for m in range(MF):
                        w1m = pw.tile([P, KH, P], dt.bfloat16, tag="w1")
                        nc.sync.dma_start(w1m[:], g1[m])
                        w3m = pw.tile([P, KH, P], dt.bfloat16, tag="w3")
                        nc.sync.dma_start(w3m[:], g3[m])
                        for j0, jn in jchunks:
                            ps1 = pps.tile([P, jn], dt.float32, tag="ps")
                            for k in range(KH):
                                nc.tensor.matmul(ps1[:], w1m[:, k, :],
                                                 xg_t[:, k, j0:j0 + jn],
                                                 start=(k == 0), stop=(k == KH - 1))
                            ps3 = pps.tile([P, jn], dt.float32, tag="ps")
                            for k in range(KH):
                                nc.tensor.matmul(ps3[:], w3m[:, k, :],
                                                 xg_t[:, k, j0:j0 + jn],
                                                 start=(k == 0), stop=(k == KH - 1))
                            sil = pev.tile([P, jn], dt.float32, tag="sil")
                            nc.scalar.activation(sil[:], ps1[:], AF.Silu)
                            nc.vector.tensor_tensor(out=gt[:, m, j0:j0 + jn],
                                                    in0=sil[:], in1=ps3[:],
                                                    op=ALU.mult)
                    if not wb_loaded:
                        # deferred off the startup critical path; first use is
                        # the first A2 eviction, hundreds of us in
                        nc.sync.dma_start(wbt[:], wb[:])
                        wb_loaded = True
                    for h in range(MH):
                        w2h = pw.tile([P, KF, P], dt.bfloat16, tag="w2")
                        nc.sync.dma_start(w2h[:], g2[h])
                        for j0, jn in jchunks:
                            ps = pps2.tile([P, jn], dt.float32, tag="ps2")
                            for k2 in range(KF):
                                nc.tensor.matmul(ps[:], w2h[:, k2, :],
                                                 gt[:, k2, j0:j0 + jn],
                                                 start=(k2 == 0), stop=(k2 == KF - 1))
                            eo = pev.tile([P, jn], dt.float32, tag="eo")
                            c0 = coff + sc0 + j0
                            nc.vector.tensor_tensor(out=eo[:], in0=ps[:],
                                                    in1=wbt[:, c0:c0 + jn],
                                                    op=ALU.mult)
                            nc.sync.dma_start(yout[h, :, c0:c0 + jn], eo[:])

    nc.finalize()
    return nc


def _fmt_lhsT(w, mtiles, ktiles):
    """[M, K] row-major -> [mtiles, P(kpart), ktiles, P(mcol)] bf16 lhsT blocks."""
    return np.ascontiguousarray(
        w.reshape(mtiles, P, ktiles, P).transpose(0, 3, 2, 1).astype(BF16))


def kernel(**inputs):
    hs = np.asarray(inputs["hidden_states"], dtype=np.float32)
    norm_w = np.asarray(inputs["norm_w"], dtype=np.float32)
    router_w = np.asarray(inputs["router_w"], dtype=np.float32)
    expert_bias = np.asarray(inputs["expert_bias"], dtype=np.float32)
    sw1 = np.asarray(inputs["sw1"], dtype=np.float32)
    sw2 = np.asarray(inputs["sw2"], dtype=np.float32)
    sw3 = np.asarray(inputs["sw3"], dtype=np.float32)
    ew1 = np.asarray(inputs["ew1"], dtype=np.float32)
    ew2 = np.asarray(inputs["ew2"], dtype=np.float32)
    ew3 = np.asarray(inputs["ew3"], dtype=np.float32)

    # ---- host: rmsnorm + router + dispatch (the "all-to-all" sharding layer)
    hflat = hs.reshape(T, H)
    ms = np.mean(hflat * hflat, axis=1, keepdims=True, dtype=np.float32)
    x = (hflat / np.sqrt(ms + EPS)) * norm_w

    logits = x @ router_w.T + expert_bias
    lmax = logits.max(axis=1, keepdims=True)
    ex = np.exp(logits - lmax, dtype=np.float32)
    probs = ex / ex.sum(axis=1, keepdims=True)
    order = np.argsort(-probs, axis=1, kind="stable")
    topi = order[:, :TOPK]
    topv = np.take_along_axis(probs, topi, axis=1)
    topv = topv / topv.sum(axis=1, keepdims=True)
    disp = np.zeros((T, E), np.float32)
    np.put_along_axis(disp, topi, topv.astype(np.float32), axis=1)

    load = probs.mean(axis=0, dtype=np.float32)
    target = np.float32(1.0 / E)
    balance = np.float32(
        np.sum(target * (np.log(target) - np.log(load))) / E * BALANCE_FACTOR)

    idxs, wvals = [], []
    for c in range(E):
        mask = (topi == c).any(axis=1)
        idx = np.nonzero(mask)[0]
        idxs.append(idx)
        wvals.append(disp[idx, c])
    maxn = max(len(i) for i in idxs)
    C = max(((maxn + 1) // 2) * 2, 32)
    CT = C + TSH

    xb = x.astype(BF16)
    # xt_host[j, p, k, t] = x[j*TSH + t, k*P + p] -- core c's shared slice is [c]
    xt_host = np.ascontiguousarray(
        xb.reshape(NCORES, TSH, KH, P).transpose(0, 3, 2, 1))

    # shared-expert weights: identical for every core, reformat once
    v1_host = _fmt_lhsT(sw1, MF, KH)
    v3_host = _fmt_lhsT(sw3, MF, KH)
    v2_host = _fmt_lhsT(sw2, MH, KF)

    in_maps = []
    for c in range(E):
        idx = idxs[c]
        pad = np.zeros(C, np.int64)
        pad[:len(idx)] = idx
        xg = xb[pad]  # [C, H] bf16 gather
        xgt_host = np.ascontiguousarray(xg.reshape(C, KH, P).transpose(2, 1, 0))
        wcol = np.ones(CT, np.float32)
        wcol[:C] = 0.0
        wcol[:len(idx)] = wvals[c]
        wb_host = np.ascontiguousarray(np.broadcast_to(wcol, (P, CT)))
        in_maps.append({
            "xs": xt_host[c],
            "xgt": xgt_host,
            "w1": _fmt_lhsT(ew1[c], MF, KH),
            "w3": _fmt_lhsT(ew3[c], MF, KH),
            "w2": _fmt_lhsT(ew2[c], MH, KF),
            "v1": v1_host,
            "v3": v3_host,
            "v2": v2_host,
            "wb": wb_host,
        })

    nc = _build_program(C)
    trace = os.environ.get("KERNEL_TRACE", "0") == "1"
    try:
        res = run_bass_kernel_spmd(nc, in_maps, core_ids=list(range(NCORES)),
                                   trace=trace)
    except Exception:
        # transient NRT_EXEC_UNIT_UNRECOVERABLE wedges resolve on retry
        res = run_bass_kernel_spmd(nc, in_maps, core_ids=list(range(NCORES)),
                                   trace=False)
    LAST_RESULTS["exec_time_ns"] = res.exec_time_ns
    LAST_RESULTS["profile_json"] = res.profile_json
    LAST_RESULTS["trace"] = res.instructions_and_trace
    LAST_RESULTS["C"] = C

    # ---- host: unshard. yout columns [0:C] = routed (pre-scaled), [C:] = shared
    out = hflat.copy()
    for c in range(E):
        y = res.results[c]["yout"].transpose(2, 0, 1).reshape(CT, H)
        out[c * TSH:(c + 1) * TSH] += y[C:]
        n = len(idxs[c])
        if n:
            out[idxs[c]] += y[:n]
    return out.reshape(B, S, H), balance
